# revision 17
# baseline (speedup 1.0000x reference)
"""Trainium2 Bass kernel for nn_DeformableBlock (deformable conv v1 block).

Contract: kernel(**inputs) takes FULL unsharded inputs, returns FULL output.
Sharding: data-parallel over batch (B=8 -> 8 NeuronCores, 1 batch each),
weights replicated.

Per-core algorithm (one batch, Cin=128, Cout=256, H=W=64):
  1. offset conv (3x3, pad 1) as 9 shifted matmuls -> offset [18, 4096]
  2. PE-transpose offsets to pixel-major [128 jp, 32 jt, 18]; quad bilinear
     arithmetic with DVE ops: base cell (by, bx) = clip(floor(py/px), 0, 62),
     one gather index idq = by*64+bx per (pixel, k), and 4 quad-cell weights
     with edge swap/select logic in both dims (matches torchvision border
     handling exactly)
  3. wrap-layout indices for dma_gather built on-chip: per (k, g) a PE matmul
     with a 0/1 group-select matrix EG produces psum[a, jt] =
     idq[g*16 + a%16, jt] (replicated to all 8 16-partition groups), DVE
     copies/casts into the int16 wrapped index tile. No DRAM round trip.
  4. xquad [4031, 512] bf16 in DRAM: row j = pixels (j, j+1, j+64, j+65)
     channel-packed, built from xT with 2 strided DRAM->DRAM copies. One
     1KB gather descriptor then fetches all 4 bilinear taps of a (pixel, k):
     dma_gather(transpose=False, elem=512) -> G [128 jp, 16 jtl, 512] bf16.
     This halves SWDGE descriptor-generation time (the Pool-engine
     bottleneck) vs pair gathers.
  5. tap weighting with per-partition scalars split across ACT (2 taps via
     activation scale) and DVE (2 scalar_tensor_tensor + 1 add); PE
     transposes the [jp, c] tile to channel-major PSUM (4 tiles per bank);
     ACT evacuates [128, 512] into val_k
  6. main conv: out[o, j] = sum_k W_k[o,:] @ val_k  (bf16 matmuls, fp32 PSUM)
  7. bias + ReLU on ScalarE, DMA out [256, 4096] f32
"""
import os
import sys
import numpy as np

try:
    import concourse.bass as bass
except ImportError:  # pragma: no cover
    sys.path.insert(0, '/opt/trn_rl_repo')
    import concourse.bass as bass
import concourse.bacc as bacc

import concourse.mybir as mybir
import concourse.tile as tile
from concourse import library_config
from concourse.bass_utils import run_bass_kernel_spmd

F32 = mybir.dt.float32
F32R = mybir.dt.float32r
BF16 = mybir.dt.bfloat16
I32 = mybir.dt.int32
I16 = mybir.dt.int16
ALU = mybir.AluOpType
ACTF = mybir.ActivationFunctionType

B, CIN, COUT, H, W = 8, 128, 256, 64, 64
HW = H * W          # 4096
NJT = HW // 128     # 32 pixel-major tiles
NK = 9
NHALF = 2
JH = HW // NHALF    # 2048 pixels per half

_CACHE = {}


def _split_multiwaits(nc, max_waits=1, kinds=None):
    """walrus CoreV3 codegen rejects control instructions carrying more
    than one sem-wait; split the excess into a chain of same-engine
    drains placed directly before the offender."""
    if kinds is None:
        kinds = (mybir.InstDrain,)
    n_split = 0
    for fn in nc.m.functions:
        for bb in fn.blocks:
            insts = list(bb.instructions)
            new = []
            changed = False
            for inst in insts:
                si = inst.sync_info
                if (isinstance(inst, kinds) and si is not None
                        and len(si.on_wait) > max_waits):
                    waits = list(si.on_wait)
                    pre, rest = waits[:-max_waits], waits[-max_waits:]
                    for i in range(0, len(pre), max_waits):
                        chunk = pre[i:i + max_waits]
                        d = mybir.InstDrain(
                            name=f"{inst.name}-wsplit{i}",
                            engine=inst.engine,
                            ins=[], outs=[],
                            sync_info=mybir.SyncInfo(
                                on_wait=chunk, on_update=[]),
                        )
                        new.append(d)
                        n_split += 1
                    inst.sync_info = mybir.SyncInfo(
                        on_wait=rest, on_update=list(si.on_update))
                    changed = True
                new.append(inst)
            if changed:
                bb.instructions = new
    return n_split


def _build_program():
    nc = bacc.Bacc('TRN2', target_bir_lowering=False, debug=False,
                   enable_asserts=False, num_devices=B)

    # ---- DRAM I/O ----
    xp_d = nc.dram_tensor('xp', [CIN, 66 * 66], F32, kind='ExternalInput')
    woffT_d = nc.dram_tensor('woffT', [9, CIN, 18], BF16, kind='ExternalInput')
    boff_d = nc.dram_tensor('boff', [18, 1], F32, kind='ExternalInput')
    wdefT_d = nc.dram_tensor('wdefT', [NK, CIN, COUT], BF16, kind='ExternalInput')
    bdef_d = nc.dram_tensor('bdef', [128, 2], F32, kind='ExternalInput')
    ident_d = nc.dram_tensor('ident', [128, 128], F32, kind='ExternalInput')
    eg_d = nc.dram_tensor('eg', [128, 8 * 128], F32, kind='ExternalInput')
    erow_d = nc.dram_tensor('erow', [128, 36 * 128], BF16, kind='ExternalInput')
    hgk_d = nc.dram_tensor('hgk', [128, NJT, NK], F32, kind='ExternalInput')
    wgk_d = nc.dram_tensor('wgk', [128, NJT, NK], F32, kind='ExternalInput')
    y_d = nc.dram_tensor('y', [COUT, HW], F32, kind='ExternalOutput')

    # DRAM scratch
    xq_d = nc.dram_tensor('xq_scratch', [HW, 4 * CIN], BF16, kind='Internal')

    with tile.TileContext(nc) as tc:
        with tc.tile_pool(name='const', bufs=1) as cpool:
            # persistent small tensors
            wdefT = cpool.tile([CIN, NK, COUT], BF16, tag='wdefT')
            nc.sync.dma_start(
                wdefT[:], wdefT_d.ap().rearrange('k c o -> c k o'))
            bdef = cpool.tile([128, 2], F32, tag='bdef')
            nc.sync.dma_start(bdef[:], bdef_d.ap())
            eg = cpool.tile([128, 8, 128], F32, tag='eg')
            nc.sync.dma_start(eg[:], eg_d.ap().rearrange('p (g a) -> p g a', g=8))
            idxwr = cpool.tile([128, NK, NHALF, 16, 8], I16, tag='idxwr')
            wnatF = cpool.tile([128, HW], BF16, tag='wnatF')
            nc.vector.memset(wnatF[:], 0)
            erow = cpool.tile([128, 36, 128], BF16, tag='erow')
            nc.sync.dma_start(
                erow[:], erow_d.ap().rearrange('p (r a) -> p r a', r=36))
            identb = cpool.tile([128, 128], BF16, tag='identb')

            # ======== phase 1: offsets, indices, weights, xquad ========
            with (
                tc.tile_pool(name='p1', bufs=1) as apool,
                tc.tile_pool(name='ps_small', bufs=2, space='PSUM') as ps_small,
                tc.tile_pool(name='ps_tr', bufs=2, space='PSUM') as ps_tr,
            ):
                x_sb = apool.tile([CIN, 66 * 66], F32, tag='x_sb')
                nc.sync.dma_start(x_sb[:], xp_d.ap())
                xbf = apool.tile([CIN, 66 * 66], BF16, tag='xbf')
                nc.vector.tensor_copy(xbf[:], x_sb[:])
                ident = apool.tile([128, 128], F32, tag='ident')
                nc.sync.dma_start(ident[:], ident_d.ap())
                woffT = apool.tile([CIN, 9, 18], BF16, tag='woffT')
                nc.sync.dma_start(
                    woffT[:], woffT_d.ap().rearrange('s c o -> c s o'))
                boff = apool.tile([18, 1], F32, tag='boff')
                nc.sync.dma_start(boff[:], boff_d.ap())
                hgk = apool.tile([128, NJT, NK], F32, tag='hgk')
                nc.sync.dma_start(hgk[:], hgk_d.ap())
                wgk = apool.tile([128, NJT, NK], F32, tag='wgk')
                nc.sync.dma_start(wgk[:], wgk_d.ap())
                nc.vector.tensor_copy(identb[:], ident[:])

                # ---------- xquad build (before conv: frees DMA early) ----
                # xc: interior [128, 4096] bf16 plus 66 zero-pad columns so
                # the shifted transpose windows below stay in bounds.
                xc = apool.tile([CIN, HW + 66], BF16, tag='xc')
                xin_all = bass.AP(
                    tensor=xbf[:].tensor,
                    offset=xbf[:].offset + 67,
                    ap=[list(xbf[:].ap[0]), [66, 64], [1, 64]])
                nc.vector.tensor_copy(xc[:, :HW], xin_all)
                nc.vector.memset(xc[:, HW:], 0)
                xTsb = apool.tile([128, NJT, CIN], BF16, tag='xTsb')
                for jt4 in range(NJT // 4):
                    ps = ps_tr.tile([128, 512], BF16, tag='psb', name='psb')
                    for q in range(4):
                        jt = jt4 * 4 + q
                        nc.tensor.transpose(
                            ps[:, 128 * q:128 * (q + 1)],
                            xc[:, 128 * jt:128 * (jt + 1)], identb[:])
                    nc.scalar.activation(
                        xTsb[:, 4 * jt4:4 * (jt4 + 1), :], ps[:], ACTF.Copy)
                # xquad row j = pixels (j, j+1, j+64, j+65) channel-packed.
                # Written straight from xTsb with partition-spread SBUF->DRAM
                # DMAs (a DRAM->DRAM copy lands on a single DMA engine; these
                # fan out across all 16). Piecewise because the +dq pixel
                # shift crosses the (jp, jt) factorization of xTsb.
                # (jp0, njp, jt0, njt, q): xq[128*(jt-jt0') + jp - dq, q] rows
                for jp0, njp, jt0, njt, q, off0 in (
                        (0, 128, 0, 32, 0, 0),
                        (1, 127, 0, 32, 1, 128),
                        (0, 1, 1, 31, 1, 127 * 512 + 128),
                        (64, 64, 0, 32, 2, 256),
                        (0, 64, 1, 31, 2, 64 * 512 + 256),
                        (65, 63, 0, 32, 3, 384),
                        (0, 65, 1, 31, 3, 63 * 512 + 384)):
                    dst = bass.AP(
                        tensor=xq_d, offset=off0,
                        ap=[[4 * CIN, njp], [128 * 4 * CIN, njt], [1, CIN]])
                    nc.sync.dma_start(
                        dst, xTsb[jp0:jp0 + njp, jt0:jt0 + njt, :])

                # ---------- offset conv: off [18, 4096] f32 ----------
                # Conv runs on the padded 66-wide grid so the streaming
                # operand is a single contiguous run; the interior is
                # extracted in the ACT epilogue.
                off_sb = apool.tile([18, HW], F32, tag='off_sb')
                chunks = [(1 + 7 * i, 7) for i in range(9)] + [(64, 1)]
                for r0, nr in chunks:
                    nfree = 66 * (nr - 1) + 64
                    ps = ps_small.tile([18, 512], F32, tag='ps', name='ps')
                    for s in range(9):
                        dh, dw = s // 3, s % 3
                        beg = r0 * 66 + 1 + (dh - 1) * 66 + (dw - 1)
                        rhs = bass.AP(
                            tensor=xbf[:].tensor,
                            offset=xbf[:].offset + beg,
                            ap=[list(xbf[:].ap[0]), [1, nfree]],
                        )
                        nc.tensor.matmul(
                            ps[:, :nfree], lhsT=woffT[:, s, :], rhs=rhs,
                            start=(s == 0), stop=(s == 8))
                    src_in = bass.AP(
                        tensor=ps[:].tensor, offset=ps[:].offset,
                        ap=[list(ps[:].ap[0]), [66, nr], [1, 64]])
                    nc.scalar.activation(
                        off_sb[:, 64 * (r0 - 1):64 * (r0 - 1 + nr)], src_in,
                        ACTF.Identity, bias=boff[:], scale=1.0)

                # ---------- transpose offsets to pixel-major ----------
                offT = apool.tile([128, NJT, 18], F32, tag='offT')
                for jt in range(NJT):
                    ps = ps_small.tile([128, 18], F32, tag='ps')
                    nc.tensor.transpose(
                        ps[:], off_sb[:, 128 * jt:128 * (jt + 1)],
                        ident[:18, :18])
                    nc.vector.tensor_copy(offT[:, jt, :], ps[:])

                # ---------- quad index/weight arithmetic (pixel-major) ----
                sh = [128, NJT, NK]

                def T(tag, dt=F32):
                    return apool.tile(sh, dt, tag=tag, name=tag)

                dyx = offT[:].rearrange('p jt (k two) -> p jt k two', two=2)
                dy = dyx[:, :, :, 0]
                dx = dyx[:, :, :, 1]

                ti = apool.tile(sh, I32, tag='ti')
                fdy, fdx = T('fdy'), T('fdx')
                tmp1, tmp2 = T('tmp1'), T('tmp2')
                # floor via int-cast trick
                nc.vector.tensor_copy(ti[:], dy)
                nc.vector.tensor_copy(fdy[:], ti[:])
                nc.vector.tensor_tensor(tmp1[:], fdy[:], dy, ALU.is_gt)
                nc.vector.tensor_tensor(fdy[:], fdy[:], tmp1[:], ALU.subtract)
                nc.vector.tensor_copy(ti[:], dx)
                nc.vector.tensor_copy(fdx[:], ti[:])
                nc.vector.tensor_tensor(tmp1[:], fdx[:], dx, ALU.is_gt)
                nc.vector.tensor_tensor(fdx[:], fdx[:], tmp1[:], ALU.subtract)

                ly, lx = T('ly'), T('lx')
                nc.vector.tensor_tensor(ly[:], dy, fdy[:], ALU.subtract)
                nc.vector.tensor_tensor(lx[:], dx, fdx[:], ALU.subtract)

                y0, x0 = T('y0'), T('x0')
                nc.vector.tensor_tensor(y0[:], hgk[:], fdy[:], ALU.add)
                nc.vector.tensor_tensor(x0[:], wgk[:], fdx[:], ALU.add)

                by, bx = T('by'), T('bx')
                nc.vector.tensor_scalar(by[:], y0[:], 62.0, 0.0, ALU.min, ALU.max)
                nc.vector.tensor_scalar(bx[:], x0[:], 62.0, 0.0, ALU.min, ALU.max)

                # edge selectors: which quad slot holds each true tap
                ey0, ey1, ey3 = T('ey0'), T('ey1'), T('ey3')
                nc.vector.tensor_tensor(ey0[:], y0[:], by[:], ALU.is_equal)
                nc.vector.tensor_scalar(tmp1[:], y0[:], 1.0, None, ALU.add)
                nc.vector.tensor_tensor(ey1[:], tmp1[:], by[:], ALU.is_equal)
                nc.vector.tensor_scalar(tmp1[:], by[:], 1.0, None, ALU.add)
                nc.vector.tensor_tensor(ey3[:], y0[:], tmp1[:], ALU.is_equal)

                ex0, ex1, ex3 = T('ex0'), T('ex1'), T('ex3')
                nc.vector.tensor_tensor(ex0[:], x0[:], bx[:], ALU.is_equal)
                nc.vector.tensor_scalar(tmp1[:], x0[:], 1.0, None, ALU.add)
                nc.vector.tensor_tensor(ex1[:], tmp1[:], bx[:], ALU.is_equal)
                nc.vector.tensor_scalar(tmp1[:], bx[:], 1.0, None, ALU.add)
                nc.vector.tensor_tensor(ex3[:], x0[:], tmp1[:], ALU.is_equal)

                # quad row/col weights: wA = e0*(1-l) + e1*l; wB = l*e0 + (1-l)*e3
                wyA, wyB = T('wyA'), T('wyB')
                nc.vector.tensor_tensor(tmp1[:], ly[:], ey0[:], ALU.mult)
                nc.vector.tensor_tensor(wyA[:], ey0[:], tmp1[:], ALU.subtract)
                nc.vector.tensor_tensor(tmp2[:], ly[:], ey1[:], ALU.mult)
                nc.vector.tensor_tensor(wyA[:], wyA[:], tmp2[:], ALU.add)
                nc.vector.tensor_tensor(tmp2[:], ly[:], ey3[:], ALU.mult)
                nc.vector.tensor_tensor(wyB[:], ey3[:], tmp2[:], ALU.subtract)
                nc.vector.tensor_tensor(wyB[:], wyB[:], tmp1[:], ALU.add)

                wxA, wxB = T('wxA'), T('wxB')
                nc.vector.tensor_tensor(tmp1[:], lx[:], ex0[:], ALU.mult)
                nc.vector.tensor_tensor(wxA[:], ex0[:], tmp1[:], ALU.subtract)
                nc.vector.tensor_tensor(tmp2[:], lx[:], ex1[:], ALU.mult)
                nc.vector.tensor_tensor(wxA[:], wxA[:], tmp2[:], ALU.add)
                nc.vector.tensor_tensor(tmp2[:], lx[:], ex3[:], ALU.mult)
                nc.vector.tensor_tensor(wxB[:], ex3[:], tmp2[:], ALU.subtract)
                nc.vector.tensor_tensor(wxB[:], wxB[:], tmp1[:], ALU.add)

                w4q = apool.tile([128, NJT, NK, 4], F32, tag='w4q')
                nc.vector.tensor_tensor(w4q[:, :, :, 0], wyA[:], wxA[:], ALU.mult)
                nc.vector.tensor_tensor(w4q[:, :, :, 1], wyA[:], wxB[:], ALU.mult)
                nc.vector.tensor_tensor(w4q[:, :, :, 2], wyB[:], wxA[:], ALU.mult)
                nc.vector.tensor_tensor(w4q[:, :, :, 3], wyB[:], wxB[:], ALU.mult)

                # gather index idq = by*64 + bx (exact ints in f32)
                idq = T('idq')
                nc.vector.tensor_scalar(tmp1[:], by[:], 64.0, None, ALU.mult)
                nc.vector.tensor_tensor(idq[:], tmp1[:], bx[:], ALU.add)

                # ---------- wrapped idx built on-chip via PE ----------
                # psum[a, jt] = idq[g*16 + a%16, jt] for all 128 partitions a:
                # one matmul per (k, g) with the 0/1 select matrix EG;
                # the mod-16 structure replicates across the 8 groups that
                # dma_gather expects. DVE casts into the int16 wrap tile.
                for k in range(NK):
                    for g in range(8):
                        ps = ps_small.tile([128, NJT], F32, tag='ps')
                        nc.tensor.matmul(
                            ps[:], lhsT=eg[:, g, :], rhs=idq[:, :, k],
                            start=True, stop=True)
                        nc.vector.tensor_copy(
                            idxwr[:, k, :, :, g], ps[:])

                # ---------- weights to j-natural rows: wnat [36, HW] ------
                # One PE transpose per jt turns w4q[:, jt, :, :] ([128, 36])
                # into psum [36, 128]; ACT casts into bf16 rows (k*4+q, j).
                for jt in range(NJT):
                    ps = ps_small.tile([36, 128], F32, tag='psw', name='ps')
                    nc.tensor.transpose(
                        ps[:],
                        w4q[:, jt, :, :].rearrange('p k q -> p (k q)'),
                        ident[:])
                    nc.scalar.activation(
                        wnatF[0:36, 128 * jt:128 * (jt + 1)], ps[:],
                        ACTF.Copy)

            # ======== phase 2: gather + weighting + conv ========
            # Channel-major: the transposing gather delivers G [c, q, j]
            # directly, so tap weights are per-COLUMN; a 1-partition PE
            # matmul (ones x weight-row) broadcasts each weight row across
            # all 128 partitions into PSUM, and the 4-tap combine is 7 bulk
            # [128, 2048] DVE ops per (k, half). No per-tile ACT ops, no
            # weighting transposes, no PSUM evacuation of val.
            with (
                tc.tile_pool(name='gath', bufs=3) as gpool,
                tc.tile_pool(name='tmp2p', bufs=3) as tpool,
                tc.tile_pool(name='val', bufs=10) as vpool,
                tc.tile_pool(name='outp', bufs=2) as opool,
                tc.tile_pool(name='ps_wb', bufs=2, space='PSUM') as ps_wb,
                tc.tile_pool(name='ps_conv', bufs=4, space='PSUM') as ps_conv,
            ):
                for half in range(NHALF):
                    j0 = half * JH
                    vals = []
                    for k in range(NK):
                        # G [128 c, 4 q, 2048 j] bf16 channel-major.
                        G = gpool.tile([128, 4, JH], BF16, tag='G', name='G')
                        in_ap = bass.AP(
                            tensor=xq_d, offset=0,
                            ap=[[4 * CIN, HW], [1, 4 * CIN]],
                        )
                        nc.gpsimd.dma_gather(
                            out_ap=G[:],
                            in_ap=in_ap,
                            idxs_ap=idxwr[:, k, half, :, :],
                            num_idxs=JH,
                            num_idxs_reg=JH,
                            elem_size=4 * CIN,
                            elem_step=4 * CIN,
                            transpose=True,
                            single_packet=False,
                        )
                        val = vpool.tile([128, JH], BF16, tag='val')
                        m0 = tpool.tile([128, JH], BF16, tag='m0', name='m0')
                        m1 = tpool.tile([128, JH], BF16, tag='m1', name='m1')
                        for q in range(4):
                            # broadcast weight row across partitions:
                            # PE (ones x row) -> f32 PSUM, ACT casts to bf16
                            ws = tpool.tile([128, JH], BF16, tag='ws',
                                            name='ws')
                            for c2 in range(2):
                                wb = ps_wb.tile([128, JH // 2], F32,
                                                tag='wb', name='wb')
                                for c4 in range(2):
                                    lo = j0 + c2 * (JH // 2) + c4 * 512
                                    nc.tensor.matmul(
                                        wb[:, c4 * 512:(c4 + 1) * 512],
                                        lhsT=erow[:, 4 * k + q, :],
                                        rhs=wnatF[:, lo:lo + 512],
                                        start=True, stop=True)
                                nc.scalar.activation(
                                    ws[:, c2 * (JH // 2):
                                       (c2 + 1) * (JH // 2)],
                                    wb[:], ACTF.Copy)
                            dst = (m0, m0, m1, m1)[q]
                            if q % 2 == 0:
                                nc.vector.tensor_tensor(
                                    dst[:], G[:, q, :], ws[:], ALU.mult)
                            else:
                                t = tpool.tile([128, JH], BF16, tag='t',
                                               name='t')
                                nc.vector.tensor_tensor(
                                    t[:], G[:, q, :], ws[:], ALU.mult)
                                nc.vector.tensor_tensor(
                                    dst[:], dst[:], t[:], ALU.add)
                        nc.vector.tensor_tensor(
                            val[:], m0[:], m1[:], ALU.add)
                        vals.append(val)

                    for jc in range(JH // 512):
                        for oh in range(2):
                            ps = ps_conv.tile([128, 512], F32, tag='ps_conv')
                            for k in range(NK):
                                nc.tensor.matmul(
                                    ps[:],
                                    lhsT=wdefT[:, k, 128 * oh:128 * (oh + 1)],
                                    rhs=vals[k][:, 512 * jc:512 * (jc + 1)],
                                    start=(k == 0), stop=(k == NK - 1))
                            yo = opool.tile([128, 512], F32, tag='yo')
                            nc.scalar.activation(
                                yo[:], ps[:], ACTF.Relu,
                                bias=bdef[:, oh:oh + 1], scale=1.0)
                            nc.sync.dma_start(
                                y_d.ap()[128 * oh:128 * (oh + 1),
                                         j0 + 512 * jc:j0 + 512 * (jc + 1)],
                                yo[:])

    nc.finalize()
    _split_multiwaits(nc)
    return nc


def _host_prep(x, w_off, b_off, w_def, b_def):
    """Build per-core input maps."""
    x = np.asarray(x, np.float32)
    w_off = np.asarray(w_off, np.float32)
    b_off = np.asarray(b_off, np.float32)
    w_def = np.asarray(w_def, np.float32)
    b_def = np.asarray(b_def, np.float32)

    woffT = np.stack([w_off[:, :, s // 3, s % 3].T for s in range(9)])
    woffT = _to_bf16(np.ascontiguousarray(woffT, np.float32))  # [9, 128, 18]
    wdefT = np.stack([w_def[:, :, s // 3, s % 3].T for s in range(9)])
    wdefT = _to_bf16(np.ascontiguousarray(wdefT))             # [9, 128, 256]
    bdef2 = np.ascontiguousarray(b_def.reshape(2, 128).T)     # [128, 2]
    ident = np.eye(128, dtype=np.float32)

    # EG[p, g, a] = 1 iff p == g*16 + (a % 16): the wrap-layout select
    # matrices (replicating each 16-partition group to all 8 groups).
    pp = np.arange(128)[:, None, None]
    gg = np.arange(8)[None, :, None]
    aa = np.arange(128)[None, None, :]
    eg = (pp == gg * 16 + (aa % 16)).astype(np.float32).reshape(128, 8 * 128)
    eg = np.ascontiguousarray(eg)
    rr = np.arange(36)[None, :, None]
    erow = (np.arange(128)[:, None, None] == rr).astype(np.float32)
    erow = _to_bf16(np.broadcast_to(erow, (128, 36, 128)).reshape(128, 36 * 128))
    erow = np.ascontiguousarray(erow)

    jp = np.arange(128)[:, None, None]
    jt = np.arange(NJT)[None, :, None]
    kk = np.arange(NK)[None, None, :]
    j = jt * 128 + jp
    ky = np.repeat(np.arange(3) - 1, 3).astype(np.float32)[kk]
    kx = np.tile(np.arange(3) - 1, 3).astype(np.float32)[kk]
    hgk = (j // 64).astype(np.float32) + ky
    wgk = (j % 64).astype(np.float32) + kx
    hgk = np.ascontiguousarray(np.broadcast_to(hgk, (128, NJT, NK)), np.float32)
    wgk = np.ascontiguousarray(np.broadcast_to(wgk, (128, NJT, NK)), np.float32)

    xp = np.pad(x, ((0, 0), (0, 0), (1, 1), (1, 1))).reshape(B, CIN, 66 * 66)

    shared = {
        'woffT': woffT,
        'boff': np.ascontiguousarray(b_off.reshape(18, 1)),
        'wdefT': wdefT,
        'bdef': bdef2,
        'ident': ident,
        'eg': eg,
        'erow': erow,
        'hgk': hgk,
        'wgk': wgk,
    }
    in_maps = []
    for b in range(B):
        m = dict(shared)
        m['xp'] = np.ascontiguousarray(xp[b])
        in_maps.append(m)
    return in_maps


def _to_bf16(a):
    import ml_dtypes
    return a.astype(ml_dtypes.bfloat16)


LAST_RESULTS = None


def _ensure_trace_support():
    """Register the NTFF profile hook that the slim agent image lacks, and
    stub out the artifact upload. Only used when KBENCH_TRACE is set."""
    import contextlib
    import ctypes
    import types

    import concourse.bass_utils as bu
    bu.upload_artifacts = lambda tmpdir: tmpdir

    if 'antenv.axon_hooks' in sys.modules:
        return
    so_path = '/opt/axon/libaxon_pjrt.so'
    if not os.path.exists(so_path):
        return
    lib = ctypes.CDLL(so_path)
    if not hasattr(lib, 'axon_start_nrt_profile'):
        return
    lib.axon_start_nrt_profile.argtypes = [
        ctypes.POINTER(ctypes.c_int64), ctypes.c_size_t]
    lib.axon_start_nrt_profile.restype = ctypes.c_int64
    lib.axon_stop_nrt_profile.argtypes = [ctypes.c_char_p]
    lib.axon_stop_nrt_profile.restype = ctypes.c_int64

    @contextlib.contextmanager
    def _hook(output_dir, device_ids):
        import jax
        jax.devices()
        if device_ids:
            ids = (ctypes.c_int64 * len(device_ids))(*device_ids)
            rc = lib.axon_start_nrt_profile(ids, len(device_ids))
        else:
            rc = lib.axon_start_nrt_profile(None, 0)
        if rc != 0:
            raise RuntimeError(f'axon_start_nrt_profile rc={rc}')
        try:
            yield
        finally:
            n = lib.axon_stop_nrt_profile(str(output_dir).encode())
            print(f'profile: {n} file(s) written to {output_dir}',
                  file=sys.stderr)

    mod = types.ModuleType('antenv.axon_hooks')
    mod.get_axon_ntff_profile_hook = lambda: _hook
    mod.set_axon_ntff_profile_hook = lambda h: None
    sys.modules['antenv.axon_hooks'] = mod


def kernel(x, w_off, b_off, w_def, b_def):
    global LAST_RESULTS
    if 'nc' not in _CACHE:
        _CACHE['nc'] = _build_program()
    nc = _CACHE['nc']
    in_maps = _host_prep(x, w_off, b_off, w_def, b_def)
    trace = bool(os.environ.get('KBENCH_TRACE'))
    if trace:
        _ensure_trace_support()
    res = run_bass_kernel_spmd(
        nc, in_maps, core_ids=list(range(B)),
        trace=trace,
    )
    LAST_RESULTS = res
    out = np.stack([res.results[b]['y'].reshape(COUT, H, W) for b in range(B)])
    return out.astype(np.float32)


# revision 20
# speedup vs baseline: 1.3757x; 1.3757x over previous
"""Trainium2 Bass kernel for nn_DeformableBlock (deformable conv v1 block).

Contract: kernel(**inputs) takes FULL unsharded inputs, returns FULL output.
Sharding: data-parallel over batch (B=8 -> 8 NeuronCores, 1 batch each),
weights replicated.

Per-core algorithm (one batch, Cin=128, Cout=256, H=W=64):
  1. offset conv (3x3, pad 1) as 9 shifted matmuls -> offset [18, 4096]
  2. PE-transpose offsets to pixel-major [128 jp, 32 jt, 18]; quad bilinear
     arithmetic with DVE ops: base cell (by, bx) = clip(floor(py/px), 0, 62),
     one gather index idq = by*64+bx per (pixel, k), and 4 quad-cell weights
     with edge swap/select logic in both dims (matches torchvision border
     handling exactly)
  3. wrap-layout indices for dma_gather built on-chip: per (k, g) a PE matmul
     with a 0/1 group-select matrix EG produces psum[a, jt] =
     idq[g*16 + a%16, jt] (replicated to all 8 16-partition groups), DVE
     copies/casts into the int16 wrapped index tile. No DRAM round trip.
  4. xquad [4031, 512] bf16 in DRAM: row j = pixels (j, j+1, j+64, j+65)
     channel-packed, built from xT with 2 strided DRAM->DRAM copies. One
     1KB gather descriptor then fetches all 4 bilinear taps of a (pixel, k):
     dma_gather(transpose=False, elem=512) -> G [128 jp, 16 jtl, 512] bf16.
     This halves SWDGE descriptor-generation time (the Pool-engine
     bottleneck) vs pair gathers.
  5. tap weighting with per-partition scalars split across ACT (2 taps via
     activation scale) and DVE (2 scalar_tensor_tensor + 1 add); PE
     transposes the [jp, c] tile to channel-major PSUM (4 tiles per bank);
     ACT evacuates [128, 512] into val_k
  6. main conv: out[o, j] = sum_k W_k[o,:] @ val_k  (bf16 matmuls, fp32 PSUM)
  7. bias + ReLU on ScalarE, DMA out [256, 4096] f32
"""
import os
import sys
import numpy as np

try:
    import concourse.bass as bass
except ImportError:  # pragma: no cover
    sys.path.insert(0, '/opt/trn_rl_repo')
    import concourse.bass as bass
import concourse.bacc as bacc

import concourse.mybir as mybir
import concourse.tile as tile
from concourse import library_config
from concourse.bass_utils import run_bass_kernel_spmd

F32 = mybir.dt.float32
F32R = mybir.dt.float32r
BF16 = mybir.dt.bfloat16
I32 = mybir.dt.int32
I16 = mybir.dt.int16
ALU = mybir.AluOpType
ACTF = mybir.ActivationFunctionType

B, CIN, COUT, H, W = 8, 128, 256, 64, 64
HW = H * W          # 4096
NJT = HW // 128     # 32 pixel-major tiles
NK = 9
NHALF = 2
JH = HW // NHALF    # 2048 pixels per half

_CACHE = {}


def _split_multiwaits(nc, max_waits=1, kinds=None):
    """walrus CoreV3 codegen rejects control instructions carrying more
    than one sem-wait; split the excess into a chain of same-engine
    drains placed directly before the offender."""
    if kinds is None:
        kinds = (mybir.InstDrain,)
    n_split = 0
    for fn in nc.m.functions:
        for bb in fn.blocks:
            insts = list(bb.instructions)
            new = []
            changed = False
            for inst in insts:
                si = inst.sync_info
                if (isinstance(inst, kinds) and si is not None
                        and len(si.on_wait) > max_waits):
                    waits = list(si.on_wait)
                    pre, rest = waits[:-max_waits], waits[-max_waits:]
                    for i in range(0, len(pre), max_waits):
                        chunk = pre[i:i + max_waits]
                        d = mybir.InstDrain(
                            name=f"{inst.name}-wsplit{i}",
                            engine=inst.engine,
                            ins=[], outs=[],
                            sync_info=mybir.SyncInfo(
                                on_wait=chunk, on_update=[]),
                        )
                        new.append(d)
                        n_split += 1
                    inst.sync_info = mybir.SyncInfo(
                        on_wait=rest, on_update=list(si.on_update))
                    changed = True
                new.append(inst)
            if changed:
                bb.instructions = new
    return n_split


def _build_program():
    nc = bacc.Bacc('TRN2', target_bir_lowering=False, debug=False,
                   enable_asserts=False, num_devices=B)

    # ---- DRAM I/O ----
    xp_d = nc.dram_tensor('xp', [CIN, 66 * 66], F32, kind='ExternalInput')
    woffT_d = nc.dram_tensor('woffT', [9, CIN, 18], BF16, kind='ExternalInput')
    boff_d = nc.dram_tensor('boff', [18, 1], F32, kind='ExternalInput')
    wdefT_d = nc.dram_tensor('wdefT', [NK, CIN, COUT], BF16, kind='ExternalInput')
    bdef_d = nc.dram_tensor('bdef', [128, 2], F32, kind='ExternalInput')
    ident_d = nc.dram_tensor('ident', [128, 128], F32, kind='ExternalInput')
    eg_d = nc.dram_tensor('eg', [128, 8 * 128], F32, kind='ExternalInput')
    hgk_d = nc.dram_tensor('hgk', [128, NJT, NK], F32, kind='ExternalInput')
    wgk_d = nc.dram_tensor('wgk', [128, NJT, NK], F32, kind='ExternalInput')
    y_d = nc.dram_tensor('y', [COUT, HW], F32, kind='ExternalOutput')

    # DRAM scratch
    xq_d = nc.dram_tensor('xq_scratch', [HW, 4 * CIN], BF16, kind='Internal')

    with tile.TileContext(nc) as tc:
        with tc.tile_pool(name='const', bufs=1) as cpool:
            # persistent small tensors
            wdefT = cpool.tile([CIN, NK, COUT], BF16, tag='wdefT')
            nc.sync.dma_start(
                wdefT[:], wdefT_d.ap().rearrange('k c o -> c k o'))
            bdef = cpool.tile([128, 2], F32, tag='bdef')
            nc.sync.dma_start(bdef[:], bdef_d.ap())
            eg = cpool.tile([128, 8, 128], F32, tag='eg')
            nc.sync.dma_start(eg[:], eg_d.ap().rearrange('p (g a) -> p g a', g=8))
            idxwr = cpool.tile([128, NK, NHALF, 16, 8], I16, tag='idxwr')
            w4q = cpool.tile([128, NJT, NK, 4], F32, tag='w4q')
            zz = cpool.tile([128, CIN], BF16, tag='zz')
            nc.vector.memset(zz[:], 0)
            identb = cpool.tile([128, 128], BF16, tag='identb')

            # ======== phase 1: offsets, indices, weights, xquad ========
            with (
                tc.tile_pool(name='p1', bufs=1) as apool,
                tc.tile_pool(name='ps_small', bufs=2, space='PSUM') as ps_small,
                tc.tile_pool(name='ps_tr', bufs=2, space='PSUM') as ps_tr,
            ):
                x_sb = apool.tile([CIN, 66 * 66], F32, tag='x_sb')
                nc.sync.dma_start(x_sb[:], xp_d.ap())
                xbf = apool.tile([CIN, 66 * 66], BF16, tag='xbf')
                nc.vector.tensor_copy(xbf[:], x_sb[:])
                ident = apool.tile([128, 128], F32, tag='ident')
                nc.sync.dma_start(ident[:], ident_d.ap())
                woffT = apool.tile([CIN, 9, 18], BF16, tag='woffT')
                nc.sync.dma_start(
                    woffT[:], woffT_d.ap().rearrange('s c o -> c s o'))
                boff = apool.tile([18, 1], F32, tag='boff')
                nc.sync.dma_start(boff[:], boff_d.ap())
                hgk = apool.tile([128, NJT, NK], F32, tag='hgk')
                nc.sync.dma_start(hgk[:], hgk_d.ap())
                wgk = apool.tile([128, NJT, NK], F32, tag='wgk')
                nc.sync.dma_start(wgk[:], wgk_d.ap())
                nc.vector.tensor_copy(identb[:], ident[:])

                # ---------- xquad build (before conv: frees DMA early) ----
                # xc: interior [128, 4096] bf16 plus 66 zero-pad columns so
                # the shifted transpose windows below stay in bounds.
                xc = apool.tile([CIN, HW + 66], BF16, tag='xc')
                xin_all = bass.AP(
                    tensor=xbf[:].tensor,
                    offset=xbf[:].offset + 67,
                    ap=[list(xbf[:].ap[0]), [66, 64], [1, 64]])
                nc.vector.tensor_copy(xc[:, :HW], xin_all)
                nc.vector.memset(xc[:, HW:], 0)
                xTsb = apool.tile([128, NJT, CIN], BF16, tag='xTsb')
                for jt4 in range(NJT // 4):
                    ps = ps_tr.tile([128, 512], BF16, tag='psb', name='psb')
                    for q in range(4):
                        jt = jt4 * 4 + q
                        nc.tensor.transpose(
                            ps[:, 128 * q:128 * (q + 1)],
                            xc[:, 128 * jt:128 * (jt + 1)], identb[:])
                    nc.scalar.activation(
                        xTsb[:, 4 * jt4:4 * (jt4 + 1), :], ps[:], ACTF.Copy)
                # xquad row j = pixels (j, j+1, j+64, j+65) channel-packed.
                # Written straight from xTsb with partition-spread SBUF->DRAM
                # DMAs (a DRAM->DRAM copy lands on a single DMA engine; these
                # fan out across all 16). Piecewise because the +dq pixel
                # shift crosses the (jp, jt) factorization of xTsb.
                # (jp0, njp, jt0, njt, q): xq[128*(jt-jt0') + jp - dq, q] rows
                for jp0, njp, jt0, njt, q, off0 in (
                        (0, 128, 0, 32, 0, 0),
                        (1, 127, 0, 32, 1, 128),
                        (0, 1, 1, 31, 1, 127 * 512 + 128),
                        (64, 64, 0, 32, 2, 256),
                        (0, 64, 1, 31, 2, 64 * 512 + 256),
                        (65, 63, 0, 32, 3, 384),
                        (0, 65, 1, 31, 3, 63 * 512 + 384)):
                    dst = bass.AP(
                        tensor=xq_d, offset=off0,
                        ap=[[4 * CIN, njp], [128 * 4 * CIN, njt], [1, CIN]])
                    nc.sync.dma_start(
                        dst, xTsb[jp0:jp0 + njp, jt0:jt0 + njt, :])

                # ---------- offset conv: off [18, 4096] f32 ----------
                # Conv runs on the padded 66-wide grid so the streaming
                # operand is a single contiguous run; the interior is
                # extracted in the ACT epilogue.
                off_sb = apool.tile([18, HW], F32, tag='off_sb')
                chunks = [(1 + 7 * i, 7) for i in range(9)] + [(64, 1)]
                for r0, nr in chunks:
                    nfree = 66 * (nr - 1) + 64
                    ps = ps_small.tile([18, 512], F32, tag='ps', name='ps')
                    for s in range(9):
                        dh, dw = s // 3, s % 3
                        beg = r0 * 66 + 1 + (dh - 1) * 66 + (dw - 1)
                        rhs = bass.AP(
                            tensor=xbf[:].tensor,
                            offset=xbf[:].offset + beg,
                            ap=[list(xbf[:].ap[0]), [1, nfree]],
                        )
                        nc.tensor.matmul(
                            ps[:, :nfree], lhsT=woffT[:, s, :], rhs=rhs,
                            start=(s == 0), stop=(s == 8))
                    src_in = bass.AP(
                        tensor=ps[:].tensor, offset=ps[:].offset,
                        ap=[list(ps[:].ap[0]), [66, nr], [1, 64]])
                    nc.scalar.activation(
                        off_sb[:, 64 * (r0 - 1):64 * (r0 - 1 + nr)], src_in,
                        ACTF.Identity, bias=boff[:], scale=1.0)

                # ---------- transpose offsets to pixel-major ----------
                offT = apool.tile([128, NJT, 18], F32, tag='offT')
                for jt in range(NJT):
                    ps = ps_small.tile([128, 18], F32, tag='ps')
                    nc.tensor.transpose(
                        ps[:], off_sb[:, 128 * jt:128 * (jt + 1)],
                        ident[:18, :18])
                    nc.vector.tensor_copy(offT[:, jt, :], ps[:])

                # ---------- quad index/weight arithmetic (pixel-major) ----
                sh = [128, NJT, NK]

                def T(tag, dt=F32):
                    return apool.tile(sh, dt, tag=tag, name=tag)

                dyx = offT[:].rearrange('p jt (k two) -> p jt k two', two=2)
                dy = dyx[:, :, :, 0]
                dx = dyx[:, :, :, 1]

                ti = apool.tile(sh, I32, tag='ti')
                fdy, fdx = T('fdy'), T('fdx')
                tmp1, tmp2 = T('tmp1'), T('tmp2')
                # floor via int-cast trick
                nc.vector.tensor_copy(ti[:], dy)
                nc.vector.tensor_copy(fdy[:], ti[:])
                nc.vector.tensor_tensor(tmp1[:], fdy[:], dy, ALU.is_gt)
                nc.vector.tensor_tensor(fdy[:], fdy[:], tmp1[:], ALU.subtract)
                nc.vector.tensor_copy(ti[:], dx)
                nc.vector.tensor_copy(fdx[:], ti[:])
                nc.vector.tensor_tensor(tmp1[:], fdx[:], dx, ALU.is_gt)
                nc.vector.tensor_tensor(fdx[:], fdx[:], tmp1[:], ALU.subtract)

                ly, lx = T('ly'), T('lx')
                nc.vector.tensor_tensor(ly[:], dy, fdy[:], ALU.subtract)
                nc.vector.tensor_tensor(lx[:], dx, fdx[:], ALU.subtract)

                y0, x0 = T('y0'), T('x0')
                nc.vector.tensor_tensor(y0[:], hgk[:], fdy[:], ALU.add)
                nc.vector.tensor_tensor(x0[:], wgk[:], fdx[:], ALU.add)

                by, bx = T('by'), T('bx')
                nc.vector.tensor_scalar(by[:], y0[:], 62.0, 0.0, ALU.min, ALU.max)
                nc.vector.tensor_scalar(bx[:], x0[:], 62.0, 0.0, ALU.min, ALU.max)

                # edge selectors: which quad slot holds each true tap
                ey0, ey1, ey3 = T('ey0'), T('ey1'), T('ey3')
                nc.vector.tensor_tensor(ey0[:], y0[:], by[:], ALU.is_equal)
                nc.vector.tensor_scalar(tmp1[:], y0[:], 1.0, None, ALU.add)
                nc.vector.tensor_tensor(ey1[:], tmp1[:], by[:], ALU.is_equal)
                nc.vector.tensor_scalar(tmp1[:], by[:], 1.0, None, ALU.add)
                nc.vector.tensor_tensor(ey3[:], y0[:], tmp1[:], ALU.is_equal)

                ex0, ex1, ex3 = T('ex0'), T('ex1'), T('ex3')
                nc.vector.tensor_tensor(ex0[:], x0[:], bx[:], ALU.is_equal)
                nc.vector.tensor_scalar(tmp1[:], x0[:], 1.0, None, ALU.add)
                nc.vector.tensor_tensor(ex1[:], tmp1[:], bx[:], ALU.is_equal)
                nc.vector.tensor_scalar(tmp1[:], bx[:], 1.0, None, ALU.add)
                nc.vector.tensor_tensor(ex3[:], x0[:], tmp1[:], ALU.is_equal)

                # quad row/col weights: wA = e0*(1-l) + e1*l; wB = l*e0 + (1-l)*e3
                wyA, wyB = T('wyA'), T('wyB')
                nc.vector.tensor_tensor(tmp1[:], ly[:], ey0[:], ALU.mult)
                nc.vector.tensor_tensor(wyA[:], ey0[:], tmp1[:], ALU.subtract)
                nc.vector.tensor_tensor(tmp2[:], ly[:], ey1[:], ALU.mult)
                nc.vector.tensor_tensor(wyA[:], wyA[:], tmp2[:], ALU.add)
                nc.vector.tensor_tensor(tmp2[:], ly[:], ey3[:], ALU.mult)
                nc.vector.tensor_tensor(wyB[:], ey3[:], tmp2[:], ALU.subtract)
                nc.vector.tensor_tensor(wyB[:], wyB[:], tmp1[:], ALU.add)

                wxA, wxB = T('wxA'), T('wxB')
                nc.vector.tensor_tensor(tmp1[:], lx[:], ex0[:], ALU.mult)
                nc.vector.tensor_tensor(wxA[:], ex0[:], tmp1[:], ALU.subtract)
                nc.vector.tensor_tensor(tmp2[:], lx[:], ex1[:], ALU.mult)
                nc.vector.tensor_tensor(wxA[:], wxA[:], tmp2[:], ALU.add)
                nc.vector.tensor_tensor(tmp2[:], lx[:], ex3[:], ALU.mult)
                nc.vector.tensor_tensor(wxB[:], ex3[:], tmp2[:], ALU.subtract)
                nc.vector.tensor_tensor(wxB[:], wxB[:], tmp1[:], ALU.add)

                nc.vector.tensor_tensor(w4q[:, :, :, 0], wyA[:], wxA[:], ALU.mult)
                nc.vector.tensor_tensor(w4q[:, :, :, 1], wyA[:], wxB[:], ALU.mult)
                nc.vector.tensor_tensor(w4q[:, :, :, 2], wyB[:], wxA[:], ALU.mult)
                nc.vector.tensor_tensor(w4q[:, :, :, 3], wyB[:], wxB[:], ALU.mult)

                # gather index idq = by*64 + bx (exact ints in f32)
                idq = T('idq')
                nc.vector.tensor_scalar(tmp1[:], by[:], 64.0, None, ALU.mult)
                nc.vector.tensor_tensor(idq[:], tmp1[:], bx[:], ALU.add)

                # ---------- wrapped idx built on-chip via PE ----------
                # psum[a, jt] = idq[g*16 + a%16, jt] for all 128 partitions a:
                # one matmul per (k, g) with the 0/1 select matrix EG;
                # the mod-16 structure replicates across the 8 groups that
                # dma_gather expects. DVE casts into the int16 wrap tile.
                for k in range(NK):
                    for g in range(8):
                        ps = ps_small.tile([128, NJT], F32, tag='ps')
                        nc.tensor.matmul(
                            ps[:], lhsT=eg[:, g, :], rhs=idq[:, :, k],
                            start=True, stop=True)
                        nc.vector.tensor_copy(
                            idxwr[:, k, :, :, g], ps[:])

            # ======== phase 2: gather + weighting + conv ========
            # Pixel-major quad gather (transpose-mode gathers pay a ~3.6x
            # DMA xbar write penalty, so weighting happens with
            # per-partition scalars). Per (k, half): 16 jtl tiles; ACT and
            # DVE split the 4-tap combine to match the ~17us/k SWDGE
            # descriptor-generation rate; PE transposes to channel-major,
            # ACT evacuates 4 tiles per PSUM bank.
            with (
                tc.tile_pool(name='gath', bufs=3) as gpool,
                tc.tile_pool(name='tmp2p', bufs=3) as tpool,
                tc.tile_pool(name='val', bufs=10) as vpool,
                tc.tile_pool(name='outp', bufs=2) as opool,
                tc.tile_pool(name='pcheck', bufs=1) as ppool,
                tc.tile_pool(name='ps_tr2', bufs=2, space='PSUM') as ps_tr2,
                tc.tile_pool(name='ps_conv', bufs=4, space='PSUM') as ps_conv,
            ):
                for half in range(NHALF):
                    j0 = half * JH
                    last = half == NHALF - 1
                    vals = []
                    parts = {}
                    for k in range(NK):
                        # G [128 jp, 16 jtl, 512] bf16: token i -> partition
                        # i%128, chunk i//128; payload = 4 taps x 128 ch.
                        G = gpool.tile([128, JH // 128, 4 * CIN], BF16,
                                       tag='G', name='G')
                        in_ap = bass.AP(
                            tensor=xq_d, offset=0,
                            ap=[[4 * CIN, HW], [1, 4 * CIN]],
                        )
                        nc.gpsimd.dma_gather(
                            out_ap=G[:],
                            in_ap=in_ap,
                            idxs_ap=idxwr[:, k, half, :, :],
                            num_idxs=JH,
                            num_idxs_reg=JH,
                            elem_size=4 * CIN,
                            elem_step=4 * CIN,
                            transpose=False,
                            single_packet=False,
                        )
                        val = vpool.tile([128, JH], BF16, tag='val')
                        for jtl4 in range(JH // 512):
                            psb = ps_tr2.tile([128, 512], BF16, tag='psb',
                                              name='psb')
                            for jj in range(4):
                                jtl = jtl4 * 4 + jj
                                jt = half * (JH // 128) + jtl
                                t0 = tpool.tile([128, CIN], BF16, tag='t0',
                                                name='t0')
                                t1 = tpool.tile([128, CIN], BF16, tag='t1',
                                                name='t1')
                                a0 = tpool.tile([128, CIN], BF16, tag='a0',
                                                name='a0')
                                a1 = tpool.tile([128, CIN], BF16, tag='a1',
                                                name='a1')
                                nc.scalar.activation(
                                    t0[:], G[:, jtl, 0:CIN], ACTF.Identity,
                                    bias=0.0,
                                    scale=w4q[:, jt, k, 0].unsqueeze(1))
                                if jtl % 4 != 3:
                                    nc.scalar.activation(
                                        t1[:], G[:, jtl, CIN:2 * CIN],
                                        ACTF.Identity, bias=0.0,
                                        scale=w4q[:, jt, k, 1].unsqueeze(1))
                                else:
                                    nc.vector.scalar_tensor_tensor(
                                        t1[:], G[:, jtl, CIN:2 * CIN],
                                        w4q[:, jt, k, 1].unsqueeze(1), zz[:],
                                        ALU.mult, ALU.add)
                                nc.vector.scalar_tensor_tensor(
                                    a0[:], G[:, jtl, 2 * CIN:3 * CIN],
                                    w4q[:, jt, k, 2].unsqueeze(1), t0[:],
                                    ALU.mult, ALU.add)
                                nc.vector.scalar_tensor_tensor(
                                    a1[:], G[:, jtl, 3 * CIN:4 * CIN],
                                    w4q[:, jt, k, 3].unsqueeze(1), t1[:],
                                    ALU.mult, ALU.add)
                                nc.vector.tensor_tensor(
                                    a0[:], a0[:], a1[:], ALU.add)
                                nc.tensor.transpose(
                                    psb[:, 128 * jj:128 * (jj + 1)],
                                    a0[:], identb[:])
                            nc.scalar.activation(
                                val[:, 512 * jtl4:512 * (jtl4 + 1)], psb[:],
                                ACTF.Copy)
                        vals.append(val)

                        # split-k partial conv for the last half: checkpoint
                        # k=0..4 into SBUF so only 4 matmuls remain per tile
                        # after the final gather.
                        if last and k == 4:
                            for jc in range(JH // 512):
                                for oh in range(2):
                                    ps = ps_conv.tile([128, 512], F32,
                                                      tag='ps_conv')
                                    for kk in range(5):
                                        nc.tensor.matmul(
                                            ps[:],
                                            lhsT=wdefT[:, kk,
                                                       128 * oh:128 * (oh + 1)],
                                            rhs=vals[kk][:, 512 * jc:
                                                         512 * (jc + 1)],
                                            start=(kk == 0), stop=(kk == 4))
                                    part = ppool.tile(
                                        [128, 512], F32,
                                        tag=f'part{jc}_{oh}',
                                        name='part')
                                    nc.scalar.activation(
                                        part[:], ps[:], ACTF.Copy)
                                    parts[(jc, oh)] = part

                    for jc in range(JH // 512):
                        for oh in range(2):
                            ps = ps_conv.tile([128, 512], F32, tag='ps_conv')
                            krange = range(5, NK) if last else range(NK)
                            for i, k in enumerate(krange):
                                nc.tensor.matmul(
                                    ps[:],
                                    lhsT=wdefT[:, k, 128 * oh:128 * (oh + 1)],
                                    rhs=vals[k][:, 512 * jc:512 * (jc + 1)],
                                    start=(i == 0), stop=(k == NK - 1))
                            yo = opool.tile([128, 512], F32, tag='yo')
                            if last:
                                nc.vector.tensor_tensor(
                                    ps[:], ps[:], parts[(jc, oh)][:], ALU.add)
                            nc.scalar.activation(
                                yo[:], ps[:], ACTF.Relu,
                                bias=bdef[:, oh:oh + 1], scale=1.0)
                            nc.sync.dma_start(
                                y_d.ap()[128 * oh:128 * (oh + 1),
                                         j0 + 512 * jc:j0 + 512 * (jc + 1)],
                                yo[:])

    nc.finalize()
    _split_multiwaits(nc)
    return nc


def _host_prep(x, w_off, b_off, w_def, b_def):
    """Build per-core input maps."""
    x = np.asarray(x, np.float32)
    w_off = np.asarray(w_off, np.float32)
    b_off = np.asarray(b_off, np.float32)
    w_def = np.asarray(w_def, np.float32)
    b_def = np.asarray(b_def, np.float32)

    woffT = np.stack([w_off[:, :, s // 3, s % 3].T for s in range(9)])
    woffT = _to_bf16(np.ascontiguousarray(woffT, np.float32))  # [9, 128, 18]
    wdefT = np.stack([w_def[:, :, s // 3, s % 3].T for s in range(9)])
    wdefT = _to_bf16(np.ascontiguousarray(wdefT))             # [9, 128, 256]
    bdef2 = np.ascontiguousarray(b_def.reshape(2, 128).T)     # [128, 2]
    ident = np.eye(128, dtype=np.float32)

    # EG[p, g, a] = 1 iff p == g*16 + (a % 16): the wrap-layout select
    # matrices (replicating each 16-partition group to all 8 groups).
    pp = np.arange(128)[:, None, None]
    gg = np.arange(8)[None, :, None]
    aa = np.arange(128)[None, None, :]
    eg = (pp == gg * 16 + (aa % 16)).astype(np.float32).reshape(128, 8 * 128)
    eg = np.ascontiguousarray(eg)

    jp = np.arange(128)[:, None, None]
    jt = np.arange(NJT)[None, :, None]
    kk = np.arange(NK)[None, None, :]
    j = jt * 128 + jp
    ky = np.repeat(np.arange(3) - 1, 3).astype(np.float32)[kk]
    kx = np.tile(np.arange(3) - 1, 3).astype(np.float32)[kk]
    hgk = (j // 64).astype(np.float32) + ky
    wgk = (j % 64).astype(np.float32) + kx
    hgk = np.ascontiguousarray(np.broadcast_to(hgk, (128, NJT, NK)), np.float32)
    wgk = np.ascontiguousarray(np.broadcast_to(wgk, (128, NJT, NK)), np.float32)

    xp = np.pad(x, ((0, 0), (0, 0), (1, 1), (1, 1))).reshape(B, CIN, 66 * 66)

    shared = {
        'woffT': woffT,
        'boff': np.ascontiguousarray(b_off.reshape(18, 1)),
        'wdefT': wdefT,
        'bdef': bdef2,
        'ident': ident,
        'eg': eg,
        'hgk': hgk,
        'wgk': wgk,
    }
    in_maps = []
    for b in range(B):
        m = dict(shared)
        m['xp'] = np.ascontiguousarray(xp[b])
        in_maps.append(m)
    return in_maps


def _to_bf16(a):
    import ml_dtypes
    return a.astype(ml_dtypes.bfloat16)


LAST_RESULTS = None


def _ensure_trace_support():
    """Register the NTFF profile hook that the slim agent image lacks, and
    stub out the artifact upload. Only used when KBENCH_TRACE is set."""
    import contextlib
    import ctypes
    import types

    import concourse.bass_utils as bu
    bu.upload_artifacts = lambda tmpdir: tmpdir

    if 'antenv.axon_hooks' in sys.modules:
        return
    so_path = '/opt/axon/libaxon_pjrt.so'
    if not os.path.exists(so_path):
        return
    lib = ctypes.CDLL(so_path)
    if not hasattr(lib, 'axon_start_nrt_profile'):
        return
    lib.axon_start_nrt_profile.argtypes = [
        ctypes.POINTER(ctypes.c_int64), ctypes.c_size_t]
    lib.axon_start_nrt_profile.restype = ctypes.c_int64
    lib.axon_stop_nrt_profile.argtypes = [ctypes.c_char_p]
    lib.axon_stop_nrt_profile.restype = ctypes.c_int64

    @contextlib.contextmanager
    def _hook(output_dir, device_ids):
        import jax
        jax.devices()
        if device_ids:
            ids = (ctypes.c_int64 * len(device_ids))(*device_ids)
            rc = lib.axon_start_nrt_profile(ids, len(device_ids))
        else:
            rc = lib.axon_start_nrt_profile(None, 0)
        if rc != 0:
            raise RuntimeError(f'axon_start_nrt_profile rc={rc}')
        try:
            yield
        finally:
            n = lib.axon_stop_nrt_profile(str(output_dir).encode())
            print(f'profile: {n} file(s) written to {output_dir}',
                  file=sys.stderr)

    mod = types.ModuleType('antenv.axon_hooks')
    mod.get_axon_ntff_profile_hook = lambda: _hook
    mod.set_axon_ntff_profile_hook = lambda h: None
    sys.modules['antenv.axon_hooks'] = mod


def kernel(x, w_off, b_off, w_def, b_def):
    global LAST_RESULTS
    if 'nc' not in _CACHE:
        _CACHE['nc'] = _build_program()
    nc = _CACHE['nc']
    in_maps = _host_prep(x, w_off, b_off, w_def, b_def)
    trace = bool(os.environ.get('KBENCH_TRACE'))
    if trace:
        _ensure_trace_support()
    res = run_bass_kernel_spmd(
        nc, in_maps, core_ids=list(range(B)),
        trace=trace,
    )
    LAST_RESULTS = res
    out = np.stack([res.results[b]['y'].reshape(COUT, H, W) for b in range(B)])
    return out.astype(np.float32)
